# revision 22
# baseline (speedup 1.0000x reference)
"""Bass/Trainium2 kernel for nn_Attention_1245540515949.

Reference computation (B=32, T=4096, H=512), fp32 inputs:
    cat    = concat([broadcast(hidden), enc], -1)          # [B,T,2H]
    energy = softmax(cat @ W_attn.T + b_attn, axis=0)      # batch-dim softmax!
    scores = relu(einsum('h,bth->bt', v, energy))[:, None] # [B,1,T]

Strategy: shard T across the 8 cores (the batch softmax stays core-local).
Per core the 512*32 = 16384 (t,b) columns (b inner) are processed in 16
blocks of 1024 columns (32 t each):

  E[h,(t,b)] = W2T.T @ enc + A'[b,h]
      bf16 matmuls, k-chunked 4x128, kc-INNER so each group of 4 accumulates
      into one PSUM bank back-to-back. A' = hidden@W1.T + b_attn is computed
      on the HOST (exact f32) and added via K=32 "indicator" matmuls; the
      four closers of an mc-pair run on four DISTINCT PE row groups
      concurrently (1 N=512 slot instead of 4). A short burst of dummy
      matmuls on memset data runs during the DMA fill so the PE power-state
      ramp completes before real data arrives.
      (fp8 DoubleRow was tried and measured: on real HW a DoubleRow N=512
      matmul costs ~2x a bf16 one (serial 256-col LDWEIGHTS), so spending
      its second K-slot on a precision residual is a wash, and pure-fp8
      packing fails the 2e-2 rel-err gate at 2.7-3%. bf16 it is.)
  X   = exp(E)
      ScalarE, one [128,1024] ACT per 2-bank PSUM tile, fp16 out.
  den[t,h] = sum_b X ; u[h,t] = v[h]/den[t,h]
      DVE segmented reduces (per-mc quarters) + reciprocal_approx_fast + mul
      per mc-half, so most of the den path overlaps the matmul stream.
  scores = u.T @ X
      per pair of blocks, 16 M=32/N=512 fp16 matmuls whose four accumulation
      chains are interleaved MM-by-MM so they run CONCURRENTLY on disjoint
      PE column groups. Issued with a 2-pair lag. Valid slots are the
      block-diagonal [32g + 16*half + jj, 32*jj + b].
  copy + DMA out (fp16); relu + diagonal extract on HOST.

enc ships as bf16 pre-arranged so each steady-state DMA moves 1 MiB with
8 KiB contiguous per partition. Pair 0 is stored j-major in DRAM and its
j0 half is split across BOTH HWDGE queues together with the w2 slices, so
block 0 is data-complete as early as the queues allow. The final block's
ACT/reduce/reciprocal run at half granularity to shorten the closing
dependency chain.
"""

import numpy as np

B, T, H = 32, 4096, 512
NCORES = 8
TC = T // NCORES          # 512 t-values per core
P = 128                   # partitions
NCOL = TC * B             # 16384 (t,b) columns per core
NBLK = NCOL // 1024       # 16 blocks of 1024 columns (32 t each)
NPAIR = NBLK // 2         # 8 block-pairs (DMA + scores-PSUM granularity)

_CACHE = {}


def _build_nc():
    import concourse.mybir as mybir
    from concourse.bacc import Bacc
    from concourse.tile import TileContext

    f32 = mybir.dt.float32
    bf16 = mybir.dt.bfloat16
    fp16 = mybir.dt.float16
    AF = mybir.ActivationFunctionType
    AX = mybir.AxisListType

    nc = Bacc()

    encb = nc.declare_dram_parameter("encb", [P, NPAIR * 8192], bf16,
                                     isOutput=False)
    w2p = nc.declare_dram_parameter("w2p", [P, 2048], bf16, isOutput=False)
    api = nc.declare_dram_parameter("api", [P, 1024], bf16, isOutput=False)
    vrep = nc.declare_dram_parameter("vrep", [P, P], f32, isOutput=False)
    out = nc.declare_dram_parameter("scores", [P, NPAIR * 512], fp16,
                                    isOutput=True)

    encv = encb.rearrange("p (pr kc j n) -> p pr kc j n", pr=NPAIR, kc=4, j=2)

    with TileContext(nc) as tc:
        with (
            tc.tile_pool(name="consts", bufs=1) as consts,
            tc.tile_pool(name="enc", bufs=NPAIR - 1) as encp,
            tc.tile_pool(name="xs", bufs=6) as xp,
            tc.tile_pool(name="dens", bufs=6) as dp,
            tc.tile_pool(name="us", bufs=6) as up,
            tc.tile_pool(name="scb", bufs=3) as scb,
            tc.tile_pool(name="eps", bufs=3, space="PSUM") as eps,
            tc.tile_pool(name="scps", bufs=2, space="PSUM") as scps,
        ):
            # ---- constants into SBUF. Pair 0 is stored j-major in DRAM
            #      (unlike the steady (kc, j, n) pairs) and the critical
            #      first-block bytes (w2 + pair-0 j0) are balanced across
            #      BOTH HWDGE queues. ----
            w2_sb = consts.tile([P, 2048], bf16, name="w2p")
            e0_sb = [consts.tile([P, 4096], bf16, name=f"enc0_{j}")
                     for j in range(2)]
            api_sb = consts.tile([P, 1024], bf16, name="api")
            vrep_sb = consts.tile([P, P], f32, name="vrep")

            # arrival order matches block-0 consumption: w2 first (gates the
            # first matmul), then pair-0 j0 half-by-half. Pair-0 DRAM layout
            # is (j, half, kc, 512).
            nc.sync.dma_start(out=w2_sb[:, 0:1024], in_=w2p[:, 0:1024])
            nc.sync.dma_start(out=e0_sb[0][:, 0:2048], in_=encb[:, 0:2048])
            nc.sync.dma_start(out=e0_sb[1][:, 0:2048], in_=encb[:, 4096:6144])
            nc.sync.dma_start(out=vrep_sb, in_=vrep[:, :])
            nc.scalar.dma_start(out=w2_sb[:, 1024:2048], in_=w2p[:, 1024:2048])
            nc.scalar.dma_start(out=e0_sb[0][:, 2048:4096],
                                in_=encb[:, 2048:4096])
            nc.scalar.dma_start(out=api_sb, in_=api[:, :])
            nc.scalar.dma_start(out=e0_sb[1][:, 2048:4096],
                                in_=encb[:, 6144:8192])
            e0v = [t_.rearrange("p (hf kc n) -> p hf kc n", hf=2, kc=4)
                   for t_ in e0_sb]

            # prewarm: exp table load on ScalarE, and a burst of dummy
            # matmuls so the PE power-state ramp completes during the DMA
            # pipeline fill instead of slowing the first real block
            warm = consts.tile([P, 512], bf16, name="warmsrc")
            nc.vector.memset(warm, 0.0)
            wact = consts.tile([1, 1], bf16, name="actwarm")
            nc.scalar.activation(out=wact, in_=warm[0:1, 0:1], func=AF.Exp)
            wps = scps.tile([P, 512], f32, tag="sc")
            for _ in range(10):
                nc.tensor.matmul(out=wps, lhsT=warm[:, 0:128], rhs=warm,
                                 start=True, stop=True,
                                 skip_group_check=True)

            # ---- main loop (scores lag 2 blocks behind the E/X pipeline
            #      so the PE instruction stream never stalls on den/u) ----
            x_hist = [None] * NBLK
            u_hist = [None] * NBLK
            for it in range(NBLK + 3):
                if it < NBLK:
                    blk = it
                    pair, j = blk // 2, blk % 2
                    if j == 0 and pair > 0:
                        etile = encp.tile([P, 8192], bf16, tag="enc")
                        for k2 in range(2):
                            nc.sync.dma_start(
                                out=etile[:, k2 * 4096:(k2 + 1) * 4096],
                                in_=encb[:, pair * 8192 + k2 * 4096:
                                         pair * 8192 + (k2 + 1) * 4096],
                            )
                        ev = etile.rearrange(
                            "p (kc j n) -> p kc j n", kc=4, j=2)
                    if pair == 0:
                        # pair-0 DRAM layout is (j, half, kc, 512)
                        def rhs_ap(kc, half, _v=e0v[j]):
                            return _v[:, half, kc]
                    else:
                        def rhs_ap(kc, half, _v=ev, _j=j):
                            return _v[:, kc, _j,
                                      half * 512:(half + 1) * 512]

                    x_all = xp.tile([P, 4096], fp16, tag="x")
                    x_hist[blk] = x_all
                    last = blk == NBLK - 1
                    # mc-pair structure: 4 consecutive kc-MMs accumulate into
                    # ONE psum bank (avoids per-MM bank cycling), and the
                    # K=32 A'-closers of two mc tiles are issued adjacently
                    # so their disjoint PE row groups overlap.
                    for mp in range(2):
                        mcs = (2 * mp, 2 * mp + 1)
                        ep_of = {}
                        for mc in mcs:
                            ep = eps.tile([P, 1024], f32, tag="e")
                            ep_of[mc] = ep
                        # half-OUTER within the mc-pair so block 0 can start
                        # on the j0-half0 data while half1 is still in flight
                        for half in range(2):
                            for mc in mcs:
                                ep = ep_of[mc]
                                for kc in range(4):
                                    nc.tensor.matmul(
                                        out=ep[:, half * 512:
                                               (half + 1) * 512],
                                        lhsT=w2_sb[:, kc * 512 + mc * P:
                                                    kc * 512 + (mc + 1) * P],
                                        rhs=rhs_ap(kc, half),
                                        start=(kc == 0), stop=False,
                                    )
                        # the aprep/ind constants are replicated in all 4
                        # 32-row groups, so each of the 4 closers of this
                        # mc-pair can use a DISTINCT PE row group -> all four
                        # run concurrently (1 N=512 slot instead of 4)
                        for half in range(2):
                            for mc in mcs:
                                rg = (mc + 2 * half) % 4
                                nc.tensor.matmul(
                                    out=ep_of[mc][:, half * 512:
                                                  (half + 1) * 512],
                                    lhsT=api_sb[32 * rg:32 * (rg + 1),
                                                mc * P:(mc + 1) * P],
                                    rhs=api_sb[32 * rg:32 * (rg + 1),
                                               512:1024],
                                    start=False, stop=True,
                                    tile_position=(32 * rg, 0),
                                )
                        for mc in mcs:
                            if last:
                                # final block: halve ACT granularity so the
                                # closing den chain starts sooner
                                for hf in range(2):
                                    nc.scalar.activation(
                                        out=x_all[:, mc * 1024 + hf * 512:
                                                  mc * 1024 + hf * 512 + 512],
                                        in_=ep_of[mc][:, hf * 512:
                                                      hf * 512 + 512],
                                        func=AF.Exp,
                                    )
                            else:
                                nc.scalar.activation(
                                    out=x_all[:, mc * 1024:(mc + 1) * 1024],
                                    in_=ep_of[mc], func=AF.Exp,
                                )

                    # den path per mc-half (the low half only needs the
                    # first mc-pair's exps, so it overlaps the second pair's
                    # matmuls and shortens the final-block tail)
                    x3 = x_all.rearrange("p (mt b) -> p mt b", b=32)
                    us = []
                    nred = 4 if last else 2
                    for hh in range(2):
                        den = dp.tile([P, 64], f32, tag=f"den{hh}")
                        for q in range(nred):
                            w = 64 // nred
                            nc.vector.reduce_sum(
                                out=den[:, q * w:(q + 1) * w],
                                in_=x3[:, hh * 64 + q * w:
                                       hh * 64 + (q + 1) * w, :],
                                axis=AX.X)
                        rden = dp.tile([P, 64], f32, tag=f"rden{hh}")
                        u = up.tile([P, 64], fp16, tag=f"u{hh}")
                        nsub = 2 if last else 1
                        for h2 in range(nsub):
                            w2_ = 64 // nsub
                            sl = slice(h2 * w2_, (h2 + 1) * w2_)
                            nc.vector.reciprocal_approx_fast(
                                out=rden[:, sl], in_=den[:, sl])
                            nc.vector.tensor_mul(
                                out=u[:, sl], in0=rden[:, sl],
                                in1=vrep_sb[:, hh * 64 + h2 * w2_:
                                            hh * 64 + (h2 + 1) * w2_])
                        us.append(u)
                    u_hist[blk] = us

                # scores for pair p at it == 2p+2: all 4 col-group chains
                # (g = 2*sj + half) interleaved MM-by-MM so they run
                # CONCURRENTLY on disjoint PE column groups -- 16 matmuls in
                # ~4-5 N=512 slots instead of 16. Valid slots are
                # out[32*g + 16*half + jj, 32*jj + b].
                if it >= 4 and (it - 4) % 2 == 0 and (it - 4) // 2 < NPAIR:
                    spair = (it - 4) // 2
                    sc_ps = scps.tile([P, 512], f32, tag="sc")
                    for mc in range(4):
                        for g in range(4):
                            sj, half = divmod(g, 2)
                            sblk = 2 * spair + sj
                            nc.tensor.matmul(
                                out=sc_ps[32 * g:32 * (g + 1), :],
                                lhsT=u_hist[sblk][mc // 2][
                                    :, (mc % 2) * 32:(mc % 2) * 32 + 32],
                                rhs=x_hist[sblk][:, mc * 1024 + half * 512:
                                                mc * 1024 + half * 512 + 512],
                                start=(mc == 0), stop=(mc == 3),
                                tile_position=(0, 32 * g),
                                skip_group_check=True,
                            )
                    ssb = scb.tile([P, 512], fp16, tag="ssb")
                    nc.vector.tensor_copy(out=ssb, in_=sc_ps)
                    nc.sync.dma_start(
                        out=out[:, spair * 512:(spair + 1) * 512],
                        in_=ssb,
                    )

    nc.compile()
    return nc


def _prep_inputs(hidden, encoder_outputs, W_attn, b_attn, v):
    """Host-side shard + layout prep. Returns in_maps for the 8 cores."""
    import ml_dtypes
    bf16 = ml_dtypes.bfloat16

    hidden = np.asarray(hidden, dtype=np.float32)
    enc = np.asarray(encoder_outputs, dtype=np.float32)
    W = np.asarray(W_attn, dtype=np.float32)
    b = np.asarray(b_attn, dtype=np.float32)
    v = np.asarray(v, dtype=np.float32)

    w2t = np.ascontiguousarray(W[:, H:].T)                   # [h_in, h_out]
    w2p = np.ascontiguousarray(
        w2t.reshape(4, P, H).transpose(1, 0, 2).reshape(P, 2048)
    ).astype(bf16)
    # A' = hidden @ W1.T + b_attn, exact on host, replicated to the 4
    # 32-row groups used by the indicator matmuls
    apr = hidden @ W[:, :H].T + b[None, :]                   # [B, H]
    aprep = np.tile(apr, (4, 1))                             # [128, 512]
    ind = np.tile(np.eye(B, dtype=np.float32), (4, 512 // B))
    api = np.concatenate([aprep, ind], axis=1).astype(bf16)  # [128, 1024]
    vcol = np.ascontiguousarray(v.reshape(4, P).T)           # [P, 4] f32
    vrep = np.repeat(vcol, 32, axis=1).astype(np.float32)    # [P, 128]

    in_maps = []
    for c in range(NCORES):
        shard = enc[c * TC:(c + 1) * TC]                     # [TC, B, H]
        encT = shard.reshape(NCOL, H).T                      # [H, NCOL]
        encb = np.ascontiguousarray(
            encT.reshape(4, P, NPAIR, 2, 1024)
                .transpose(1, 2, 0, 3, 4).reshape(P, NPAIR * 8192)
        )
        # pair 0 is stored (j, half, kc, 512): each j-half-of-columns is one
        # contiguous DMA whose arrival order matches matmul consumption
        encb[:, :8192] = np.ascontiguousarray(
            encb[:, :8192].reshape(P, 4, 2, 2, 512).transpose(0, 2, 3, 1, 4)
            .reshape(P, 8192))
        in_maps.append({
            "encb": encb.astype(bf16), "w2p": w2p, "api": api, "vrep": vrep,
        })
    return in_maps


def _assemble(results):
    """results: per-core dicts with 'scores' [128, NPAIR*512] fp16.

    Column layout: col = pair*512 + 32*jj + b. Valid rows per quarter q
    (t = 64*pair + 16*q + jj): q=0 -> row jj, q=1 -> 48+jj, q=2 -> 64+jj,
    q=3 -> 112+jj.
    """
    rowbase = (0, 48, 64, 112)
    out = np.empty((B, 1, T), np.float32)
    for c in range(NCORES):
        s = np.asarray(results[c]["scores"], dtype=np.float32)
        s4 = s.reshape(P, NPAIR, 16, B)                      # [row,pair,jj,b]
        for q in range(4):
            for jj in range(16):
                vals = s4[rowbase[q] + jj, :, jj, :]         # [pair, b]
                t0 = c * TC + 16 * q + jj
                out[:, 0, t0:t0 + 64 * NPAIR:64] = np.maximum(vals, 0.0).T
    return out


def run(in_maps, trace=False, **kw):
    from concourse.bass_utils import run_bass_kernel_spmd

    if "nc" not in _CACHE:
        _CACHE["nc"] = _build_nc()
    nc = _CACHE["nc"]
    return run_bass_kernel_spmd(
        nc, in_maps, list(range(NCORES)), trace=trace, **kw
    )


def kernel(hidden, encoder_outputs, W_attn, b_attn, v):
    in_maps = _prep_inputs(hidden, encoder_outputs, W_attn, b_attn, v)
    br = run(in_maps)
    return _assemble(br.results)


# revision 23
# speedup vs baseline: 1.0421x; 1.0421x over previous
"""Bass/Trainium2 kernel for nn_Attention_1245540515949.

Reference computation (B=32, T=4096, H=512), fp32 inputs:
    cat    = concat([broadcast(hidden), enc], -1)          # [B,T,2H]
    energy = softmax(cat @ W_attn.T + b_attn, axis=0)      # batch-dim softmax!
    scores = relu(einsum('h,bth->bt', v, energy))[:, None] # [B,1,T]

Strategy: shard T across the 8 cores (the batch softmax stays core-local).
Per core the 512*32 = 16384 (t,b) columns (b inner) are processed in 16
blocks of 1024 columns (32 t each):

  E[h,(t,b)] = W2T.T @ enc + A'[b,h]
      bf16 matmuls, k-chunked 4x128, kc-INNER so each group of 4 accumulates
      into one PSUM bank back-to-back. A' = hidden@W1.T + b_attn is computed
      on the HOST (exact f32) and added via K=32 "indicator" matmuls; the
      four closers of an mc-pair run on four DISTINCT PE row groups
      concurrently (1 N=512 slot instead of 4). A short burst of dummy
      matmuls on memset data runs during the DMA fill so the PE power-state
      ramp completes before real data arrives.
      (fp8 DoubleRow was tried and measured: on real HW a DoubleRow N=512
      matmul costs ~2x a bf16 one (serial 256-col LDWEIGHTS), so spending
      its second K-slot on a precision residual is a wash, and pure-fp8
      packing fails the 2e-2 rel-err gate at 2.7-3%. bf16 it is.)
  X   = exp(E)
      ScalarE, one [128,1024] ACT per 2-bank PSUM tile, fp16 out.
  den[t,h] = sum_b X ; u[h,t] = v[h]/den[t,h]
      DVE segmented reduces (per-mc quarters) + reciprocal_approx_fast + mul
      per mc-half, so most of the den path overlaps the matmul stream.
  scores = u.T @ X
      per pair of blocks, 16 M=32/N=512 fp16 matmuls whose four accumulation
      chains are interleaved MM-by-MM so they run CONCURRENTLY on disjoint
      PE column groups. Issued with a 2-pair lag. Valid slots are the
      block-diagonal [32g + 16*half + jj, 32*jj + b].
  copy + DMA out (fp16); relu + diagonal extract on HOST.

enc ships as bf16 pre-arranged so each steady-state DMA moves 1 MiB with
8 KiB contiguous per partition. Pair 0 is stored j-major in DRAM and its
j0 half is split across BOTH HWDGE queues together with the w2 slices, so
block 0 is data-complete as early as the queues allow. The final block's
ACT/reduce/reciprocal run at half granularity to shorten the closing
dependency chain.
"""

import numpy as np

B, T, H = 32, 4096, 512
NCORES = 8
TC = T // NCORES          # 512 t-values per core
P = 128                   # partitions
NCOL = TC * B             # 16384 (t,b) columns per core
NBLK = NCOL // 1024       # 16 blocks of 1024 columns (32 t each)
NPAIR = NBLK // 2         # 8 block-pairs (DMA + scores-PSUM granularity)

_CACHE = {}


def _build_nc():
    import concourse.mybir as mybir
    from concourse.bacc import Bacc
    from concourse.tile import TileContext

    f32 = mybir.dt.float32
    bf16 = mybir.dt.bfloat16
    fp16 = mybir.dt.float16
    AF = mybir.ActivationFunctionType
    AX = mybir.AxisListType

    nc = Bacc()

    encb = nc.declare_dram_parameter("encb", [P, NPAIR * 8192], bf16,
                                     isOutput=False)
    w2p = nc.declare_dram_parameter("w2p", [P, 2048], bf16, isOutput=False)
    api = nc.declare_dram_parameter("api", [P, 1024], bf16, isOutput=False)
    vrep = nc.declare_dram_parameter("vrep", [P, P], f32, isOutput=False)
    out = nc.declare_dram_parameter("scores", [P, NPAIR * 512], fp16,
                                    isOutput=True)

    encv = encb.rearrange("p (pr kc j n) -> p pr kc j n", pr=NPAIR, kc=4, j=2)

    with TileContext(nc) as tc:
        with (
            tc.tile_pool(name="consts", bufs=1) as consts,
            tc.tile_pool(name="enc", bufs=NPAIR - 1) as encp,
            tc.tile_pool(name="xs", bufs=6) as xp,
            tc.tile_pool(name="dens", bufs=6) as dp,
            tc.tile_pool(name="us", bufs=6) as up,
            tc.tile_pool(name="scb", bufs=3) as scb,
            tc.tile_pool(name="eps", bufs=3, space="PSUM") as eps,
            tc.tile_pool(name="scps", bufs=2, space="PSUM") as scps,
        ):
            # ---- constants into SBUF. Pair 0 is stored j-major in DRAM
            #      (unlike the steady (kc, j, n) pairs) and the critical
            #      first-block bytes (w2 + pair-0 j0) are balanced across
            #      BOTH HWDGE queues. ----
            w2_sb = consts.tile([P, 2048], bf16, name="w2p")
            e0_sb = [consts.tile([P, 4096], bf16, name=f"enc0_{j}")
                     for j in range(2)]
            api_sb = consts.tile([P, 1024], bf16, name="api")
            vrep_sb = consts.tile([P, P], f32, name="vrep")

            # arrival order matches block-0 consumption: w2 first (gates the
            # first matmul), then pair-0 j0 half-by-half. Pair-0 DRAM layout
            # is (j, half, kc, 512).
            nc.sync.dma_start(out=w2_sb[:, 0:1024], in_=w2p[:, 0:1024])
            nc.sync.dma_start(out=e0_sb[0][:, 0:2048], in_=encb[:, 0:2048])
            nc.sync.dma_start(out=e0_sb[1][:, 0:2048], in_=encb[:, 4096:6144])
            nc.sync.dma_start(out=vrep_sb, in_=vrep[:, :])
            nc.scalar.dma_start(out=w2_sb[:, 1024:2048], in_=w2p[:, 1024:2048])
            nc.scalar.dma_start(out=e0_sb[0][:, 2048:4096],
                                in_=encb[:, 2048:4096])
            nc.scalar.dma_start(out=api_sb, in_=api[:, :])
            nc.scalar.dma_start(out=e0_sb[1][:, 2048:4096],
                                in_=encb[:, 6144:8192])
            e0v = [t_.rearrange("p (hf kc n) -> p hf kc n", hf=2, kc=4)
                   for t_ in e0_sb]

            # prewarm: exp table load on ScalarE, and a burst of dummy
            # matmuls so the PE power-state ramp completes during the DMA
            # pipeline fill instead of slowing the first real block
            warm = consts.tile([P, 512], bf16, name="warmsrc")
            nc.vector.memset(warm, 0.0)
            wact = consts.tile([1, 1], bf16, name="actwarm")
            nc.scalar.activation(out=wact, in_=warm[0:1, 0:1], func=AF.Exp)
            wps = scps.tile([P, 512], f32, tag="sc")
            for _ in range(10):
                nc.tensor.matmul(out=wps, lhsT=warm[:, 0:128], rhs=warm,
                                 start=True, stop=True,
                                 skip_group_check=True)

            # ---- main loop (scores lag 2 blocks behind the E/X pipeline
            #      so the PE instruction stream never stalls on den/u) ----
            x_hist = [None] * NBLK
            u_hist = [None] * NBLK
            for it in range(NBLK + 3):
                if it < NBLK:
                    blk = it
                    pair, j = blk // 2, blk % 2
                    if j == 0 and pair > 0:
                        etile = encp.tile([P, 8192], bf16, tag="enc")
                        for k2 in range(2):
                            nc.sync.dma_start(
                                out=etile[:, k2 * 4096:(k2 + 1) * 4096],
                                in_=encb[:, pair * 8192 + k2 * 4096:
                                         pair * 8192 + (k2 + 1) * 4096],
                            )
                        ev = etile.rearrange(
                            "p (kc j n) -> p kc j n", kc=4, j=2)
                    if pair == 0:
                        # pair-0 DRAM layout is (j, half, kc, 512)
                        def rhs_ap(kc, half, _v=e0v[j]):
                            return _v[:, half, kc]
                    else:
                        def rhs_ap(kc, half, _v=ev, _j=j):
                            return _v[:, kc, _j,
                                      half * 512:(half + 1) * 512]

                    x_all = xp.tile([P, 4096], fp16, tag="x")
                    x_hist[blk] = x_all
                    last = blk == NBLK - 1
                    # mc-pair structure: 4 consecutive kc-MMs accumulate into
                    # ONE psum bank (avoids per-MM bank cycling), and the
                    # K=32 A'-closers of two mc tiles are issued adjacently
                    # so their disjoint PE row groups overlap.
                    for mp in range(2):
                        mcs = (2 * mp, 2 * mp + 1)
                        ep_of = {}
                        for mc in mcs:
                            ep = eps.tile([P, 1024], f32, tag="e")
                            ep_of[mc] = ep
                        # block 0 only: half-OUTER within the mc-pair so it
                        # can start on the j0-half0 data while half1 is in
                        # flight. Steady blocks stay mc-outer: half-outer
                        # keeps both psum tiles open across the whole pair,
                        # and the eps pool (bufs=3) then stalls every group
                        # start ~0.45us waiting on the draining ACT.
                        if blk == 0:
                            mh = [(h, mc) for h in range(2) for mc in mcs]
                        else:
                            mh = [(h, mc) for mc in mcs for h in range(2)]
                        for half, mc in mh:
                            ep = ep_of[mc]
                            for kc in range(4):
                                nc.tensor.matmul(
                                    out=ep[:, half * 512:
                                           (half + 1) * 512],
                                    lhsT=w2_sb[:, kc * 512 + mc * P:
                                                kc * 512 + (mc + 1) * P],
                                    rhs=rhs_ap(kc, half),
                                    start=(kc == 0), stop=False,
                                )
                        # the aprep/ind constants are replicated in all 4
                        # 32-row groups, so each of the 4 closers of this
                        # mc-pair can use a DISTINCT PE row group -> all four
                        # run concurrently (1 N=512 slot instead of 4)
                        for half in range(2):
                            for mc in mcs:
                                rg = (mc + 2 * half) % 4
                                nc.tensor.matmul(
                                    out=ep_of[mc][:, half * 512:
                                                  (half + 1) * 512],
                                    lhsT=api_sb[32 * rg:32 * (rg + 1),
                                                mc * P:(mc + 1) * P],
                                    rhs=api_sb[32 * rg:32 * (rg + 1),
                                               512:1024],
                                    start=False, stop=True,
                                    tile_position=(32 * rg, 0),
                                )
                        for mc in mcs:
                            if last:
                                # final block: halve ACT granularity so the
                                # closing den chain starts sooner
                                for hf in range(2):
                                    nc.scalar.activation(
                                        out=x_all[:, mc * 1024 + hf * 512:
                                                  mc * 1024 + hf * 512 + 512],
                                        in_=ep_of[mc][:, hf * 512:
                                                      hf * 512 + 512],
                                        func=AF.Exp,
                                    )
                            else:
                                nc.scalar.activation(
                                    out=x_all[:, mc * 1024:(mc + 1) * 1024],
                                    in_=ep_of[mc], func=AF.Exp,
                                )

                    # den path per mc-half (the low half only needs the
                    # first mc-pair's exps, so it overlaps the second pair's
                    # matmuls and shortens the final-block tail)
                    x3 = x_all.rearrange("p (mt b) -> p mt b", b=32)
                    us = []
                    nred = 4 if last else 2
                    for hh in range(2):
                        den = dp.tile([P, 64], f32, tag=f"den{hh}")
                        for q in range(nred):
                            w = 64 // nred
                            nc.vector.reduce_sum(
                                out=den[:, q * w:(q + 1) * w],
                                in_=x3[:, hh * 64 + q * w:
                                       hh * 64 + (q + 1) * w, :],
                                axis=AX.X)
                        rden = dp.tile([P, 64], f32, tag=f"rden{hh}")
                        u = up.tile([P, 64], fp16, tag=f"u{hh}")
                        nsub = 2 if last else 1
                        for h2 in range(nsub):
                            w2_ = 64 // nsub
                            sl = slice(h2 * w2_, (h2 + 1) * w2_)
                            nc.vector.reciprocal_approx_fast(
                                out=rden[:, sl], in_=den[:, sl])
                            nc.vector.tensor_mul(
                                out=u[:, sl], in0=rden[:, sl],
                                in1=vrep_sb[:, hh * 64 + h2 * w2_:
                                            hh * 64 + (h2 + 1) * w2_])
                        us.append(u)
                    u_hist[blk] = us

                # scores for pair p at it == 2p+2: all 4 col-group chains
                # (g = 2*sj + half) interleaved MM-by-MM so they run
                # CONCURRENTLY on disjoint PE column groups -- 16 matmuls in
                # ~4-5 N=512 slots instead of 16. Valid slots are
                # out[32*g + 16*half + jj, 32*jj + b].
                if it >= 4 and (it - 4) % 2 == 0 and (it - 4) // 2 < NPAIR:
                    spair = (it - 4) // 2
                    sc_ps = scps.tile([P, 512], f32, tag="sc")
                    for mc in range(4):
                        for g in range(4):
                            sj, half = divmod(g, 2)
                            sblk = 2 * spair + sj
                            nc.tensor.matmul(
                                out=sc_ps[32 * g:32 * (g + 1), :],
                                lhsT=u_hist[sblk][mc // 2][
                                    :, (mc % 2) * 32:(mc % 2) * 32 + 32],
                                rhs=x_hist[sblk][:, mc * 1024 + half * 512:
                                                mc * 1024 + half * 512 + 512],
                                start=(mc == 0), stop=(mc == 3),
                                tile_position=(0, 32 * g),
                                skip_group_check=True,
                            )
                    ssb = scb.tile([P, 512], fp16, tag="ssb")
                    nc.vector.tensor_copy(out=ssb, in_=sc_ps)
                    nc.sync.dma_start(
                        out=out[:, spair * 512:(spair + 1) * 512],
                        in_=ssb,
                    )

    nc.compile()
    return nc


def _prep_inputs(hidden, encoder_outputs, W_attn, b_attn, v):
    """Host-side shard + layout prep. Returns in_maps for the 8 cores."""
    import ml_dtypes
    bf16 = ml_dtypes.bfloat16

    hidden = np.asarray(hidden, dtype=np.float32)
    enc = np.asarray(encoder_outputs, dtype=np.float32)
    W = np.asarray(W_attn, dtype=np.float32)
    b = np.asarray(b_attn, dtype=np.float32)
    v = np.asarray(v, dtype=np.float32)

    w2t = np.ascontiguousarray(W[:, H:].T)                   # [h_in, h_out]
    w2p = np.ascontiguousarray(
        w2t.reshape(4, P, H).transpose(1, 0, 2).reshape(P, 2048)
    ).astype(bf16)
    # A' = hidden @ W1.T + b_attn, exact on host, replicated to the 4
    # 32-row groups used by the indicator matmuls
    apr = hidden @ W[:, :H].T + b[None, :]                   # [B, H]
    aprep = np.tile(apr, (4, 1))                             # [128, 512]
    ind = np.tile(np.eye(B, dtype=np.float32), (4, 512 // B))
    api = np.concatenate([aprep, ind], axis=1).astype(bf16)  # [128, 1024]
    vcol = np.ascontiguousarray(v.reshape(4, P).T)           # [P, 4] f32
    vrep = np.repeat(vcol, 32, axis=1).astype(np.float32)    # [P, 128]

    in_maps = []
    for c in range(NCORES):
        shard = enc[c * TC:(c + 1) * TC]                     # [TC, B, H]
        encT = shard.reshape(NCOL, H).T                      # [H, NCOL]
        encb = np.ascontiguousarray(
            encT.reshape(4, P, NPAIR, 2, 1024)
                .transpose(1, 2, 0, 3, 4).reshape(P, NPAIR * 8192)
        )
        # pair 0 is stored (j, half, kc, 512): each j-half-of-columns is one
        # contiguous DMA whose arrival order matches matmul consumption
        encb[:, :8192] = np.ascontiguousarray(
            encb[:, :8192].reshape(P, 4, 2, 2, 512).transpose(0, 2, 3, 1, 4)
            .reshape(P, 8192))
        in_maps.append({
            "encb": encb.astype(bf16), "w2p": w2p, "api": api, "vrep": vrep,
        })
    return in_maps


def _assemble(results):
    """results: per-core dicts with 'scores' [128, NPAIR*512] fp16.

    Column layout: col = pair*512 + 32*jj + b. Valid rows per quarter q
    (t = 64*pair + 16*q + jj): q=0 -> row jj, q=1 -> 48+jj, q=2 -> 64+jj,
    q=3 -> 112+jj.
    """
    rowbase = (0, 48, 64, 112)
    out = np.empty((B, 1, T), np.float32)
    for c in range(NCORES):
        s = np.asarray(results[c]["scores"], dtype=np.float32)
        s4 = s.reshape(P, NPAIR, 16, B)                      # [row,pair,jj,b]
        for q in range(4):
            for jj in range(16):
                vals = s4[rowbase[q] + jj, :, jj, :]         # [pair, b]
                t0 = c * TC + 16 * q + jj
                out[:, 0, t0:t0 + 64 * NPAIR:64] = np.maximum(vals, 0.0).T
    return out


def run(in_maps, trace=False, **kw):
    from concourse.bass_utils import run_bass_kernel_spmd

    if "nc" not in _CACHE:
        _CACHE["nc"] = _build_nc()
    nc = _CACHE["nc"]
    return run_bass_kernel_spmd(
        nc, in_maps, list(range(NCORES)), trace=trace, **kw
    )


def kernel(hidden, encoder_outputs, W_attn, b_attn, v):
    in_maps = _prep_inputs(hidden, encoder_outputs, W_attn, b_attn, v)
    br = run(in_maps)
    return _assemble(br.results)
